# revision 2
# baseline (speedup 1.0000x reference)
"""AttentionWithRotary on 8 Trainium2 cores — v2 (restructured).

Math (no softmax, associativity): per head
    out_h = Q_r @ (K_r^T @ V) / sqrt(hd)
Sharding: data-parallel batch (2) x tensor-parallel heads (4/core).
Partials summed on host; wo bias added there.

v2 structure (vs v1): phases separated (Q | KV+Mh | N | out) with all of
x resident so stationary weights are reused across long moving streams:
  Q:  stationary wq[k,h] serves a blk-PAIR (1024 cols)   -> 128 ldweights
  KV: stationary x[k,m] serves K and V (1024 cols)       -> 256
  Mh: 64; N: stationary m_sb[h] serves 4 chunks          -> 4
  out: stationary qtr[h,m-slice] serves 4 n-chunks       -> 64
(vs ~1100 ldweights in v1 — PE queue dispatch is ~40ns/instr.)
Rope chains emit bf16 intermediates (2x DVE rate); K bias is added via a
broadcast tile (no 4MB rbk stream); outputs are bf16 partials.
"""
import numpy as np
import ml_dtypes
from contextlib import ExitStack

import concourse.bacc as bacc
import concourse.tile as tile
import concourse.mybir as mybir
from concourse.bass_utils import run_bass_kernel_spmd

BF16 = mybir.dt.bfloat16
F32 = mybir.dt.float32
NPBF = ml_dtypes.bfloat16

S = 2048
DIN = 2048
NH = 16
HD = 128
B = 2
NCORES = 8
TP = 4                 # head-parallel ways
NHL = NH // TP         # 4 heads per core
DLOC = NHL * HD        # 512 local head dims
BLK = 512
NBLK = S // BLK        # 4 seq blocks
KT = DIN // 128        # 16 contraction tiles
P = 128

_NC_CACHE = None


ABLATE_ROPE = False  # timing experiment: replace rope chains with copies


def _emit(nc, tc, ctx, d, out_d, reps=1, loop_n=0):
    ADD = mybir.AluOpType.add
    MULT = mybir.AluOpType.mult

    wpool = ctx.enter_context(tc.tile_pool(name="w", bufs=1))
    tpool = ctx.enter_context(tc.tile_pool(name="trig", bufs=3))
    qpool = ctx.enter_context(tc.tile_pool(name="qtr", bufs=1))
    kvpool = ctx.enter_context(tc.tile_pool(name="kv", bufs=3))
    tmp = ctx.enter_context(tc.tile_pool(name="tmp", bufs=2))
    osb = ctx.enter_context(tc.tile_pool(name="osb", bufs=6))
    psum = ctx.enter_context(tc.tile_pool(name="ps", bufs=7, space="PSUM"))
    mps = ctx.enter_context(tc.tile_pool(name="mps", bufs=1, space="PSUM"))

    # --- resident constants, in consumption order ---
    bqc = wpool.tile([P, NHL], BF16, name="bqc", tag="bqc")
    nc.sync.dma_start(bqc[:], d["bqc"][:])
    bk_rep = wpool.tile([P, DLOC], BF16, name="bk_rep", tag="bk_rep")
    nc.sync.dma_start(bk_rep[:], d["bk_rep"][:])
    bv_rep = wpool.tile([P, DLOC], BF16, name="bv_rep", tag="bv_rep")
    nc.sync.dma_start(bv_rep[:], d["bv_rep"][:])
    c2 = wpool.tile([P, S], BF16, name="c2", tag="c2")
    nc.sync.dma_start(c2[:], d["c2"][:])
    s2x = wpool.tile([P, S], BF16, name="s2x", tag="s2x")
    nc.sync.dma_start(s2x[:], d["s2x"][:])
    wq_all = wpool.tile([P, KT * DLOC], BF16, name="wq_all", tag="wq_all")
    for k in range(KT):
        nc.sync.dma_start(wq_all[:, k * DLOC:(k + 1) * DLOC],
                          d["wqT"][k * 128:(k + 1) * 128, :])
    wk_all = wpool.tile([P, KT * DLOC], BF16, name="wk_all", tag="wk_all")
    wv_all = wpool.tile([P, KT * DLOC], BF16, name="wv_all", tag="wv_all")
    for k in range(KT):
        nc.sync.dma_start(wk_all[:, k * DLOC:(k + 1) * DLOC],
                          d["wkT"][k * 128:(k + 1) * 128, :])
        nc.sync.dma_start(wv_all[:, k * DLOC:(k + 1) * DLOC],
                          d["wvT"][k * 128:(k + 1) * 128, :])

    # x fully resident: xt[blk] [128, KT*512]
    xt = [wpool.tile([P, KT * BLK], BF16, name=f"xt{b}", tag=f"xt{b}")
          for b in range(NBLK)]

    qtr = [qpool.tile([P, S], BF16, name=f"qtr{h}", tag=f"qtr{h}")
           for h in range(NHL)]

    def pair_view(t):
        return t.rearrange("p (h u j) -> p h u j", h=NHL, u=2, j=64)

    # wo reuses wq_all's SBUF (same size); the DMA naturally waits for the
    # last Q matmul via tile WAR tracking.
    wo_all = wq_all

    def body(first):
        # x loads: blk 0,2 on the Pool (gpsimd) queue, blk 1,3 on the Act
        # (scalar) queue so the first Q-slot's pair arrives at 2 tiles/us.
        # per-k 2D DMAs (3D k-batched APs are ~30x slower on hardware).
        for blk in (0, 2):
            for k in range(KT):
                nc.gpsimd.dma_start(
                    xt[blk][:, k * BLK:(k + 1) * BLK],
                    d["xT"][k * 128:(k + 1) * 128, blk * BLK:(blk + 1) * BLK])
        for blk in (1, 3):
            for k in range(KT):
                nc.scalar.dma_start(
                    xt[blk][:, k * BLK:(k + 1) * BLK],
                    d["xT"][k * 128:(k + 1) * 128, blk * BLK:(blk + 1) * BLK])

        # --- phase A: Q projection + rope, blk-pairs share stationary ---
        for pr in range(2):
            b0, b1 = 2 * pr, 2 * pr + 1
            for h in range(NHL):
                h0, h1 = h * 128, (h + 1) * 128
                qp = [psum.tile([P, BLK], F32, name="q_ps", tag="ps")
                      for _ in range(2)]
                for k in range(KT):
                    for i, blk in enumerate((b0, b1)):
                        nc.tensor.matmul(
                            qp[i][:],
                            wq_all[:, k * DLOC + h0:k * DLOC + h1],
                            xt[blk][:, k * BLK:(k + 1) * BLK],
                            start=(k == 0), stop=(k == KT - 1),
                            skip_group_check=True)
                for i, blk in enumerate((b0, b1)):
                    c0, c1 = blk * BLK, (blk + 1) * BLK
                    if ABLATE_ROPE:
                        nc.scalar.copy(qtr[h][:, c0:c1], qp[i][:])
                        continue
                    a = tmp.tile([P, BLK], BF16, name="a", tag="t0")
                    nc.vector.scalar_tensor_tensor(
                        a[:], qp[i][:], bqc[:, h:h + 1], c2[:, c0:c1],
                        ADD, MULT)
                    bb = tmp.tile([P, BLK], BF16, name="bb", tag="t1")
                    nc.vector.scalar_tensor_tensor(
                        bb[0:64, :], qp[i][64:128, :], bqc[64:128, h:h + 1],
                        s2x[64:128, c0:c1], ADD, MULT)
                    nc.vector.scalar_tensor_tensor(
                        bb[64:128, :], qp[i][0:64, :], bqc[0:64, h:h + 1],
                        s2x[0:64, c0:c1], ADD, MULT)
                    nc.vector.tensor_add(qtr[h][:, c0:c1], a[:], bb[:])

        # wo overwrites wq_all's SBUF (tile WAR tracking delays the DMA
        # until the last Q matmul has read wq)
        for h in range(NHL):
            nc.sync.dma_start(wo_all[:, h * DIN:(h + 1) * DIN],
                              d["woT"][h * 128:(h + 1) * 128, :])

        # --- phase B: K,V projections (shared stationary) + Mh accum ---
        mh_ps = mps.tile([P, DLOC], F32, name="mh_ps", tag="mh")

        def emit_mh(kr, vt, mg):
            if mg == 0:
                # start=True clears the whole PSUM bank; the critical
                # section pins the order of first-writes within the bank.
                with tc.tile_critical():
                    for h in range(NHL):
                        h0, h1 = h * 128, (h + 1) * 128
                        nc.tensor.matmul(mh_ps[:, h0:h1], vt[:, h0:h1],
                                         kr[:, h0:h1], start=(h == 0),
                                         stop=False, skip_group_check=True)
            else:
                for h in range(NHL):
                    h0, h1 = h * 128, (h + 1) * 128
                    nc.tensor.matmul(mh_ps[:, h0:h1], vt[:, h0:h1],
                                     kr[:, h0:h1], start=False,
                                     stop=(mg == 4 * NBLK - 1),
                                     skip_group_check=True)

        mh_pend = None
        for blk in range(NBLK):
            for m in range(4):
                mg = blk * 4 + m
                r0 = mg * 128
                k_ps = psum.tile([P, DLOC], F32, name="k_ps", tag="ps")
                v_ps = psum.tile([P, DLOC], F32, name="v_ps", tag="ps")
                for k in range(KT):
                    xlk = xt[blk][:, k * BLK + m * 128:k * BLK + (m + 1) * 128]
                    nc.tensor.matmul(k_ps[:], xlk,
                                     wk_all[:, k * DLOC:(k + 1) * DLOC],
                                     start=(k == 0), stop=(k == KT - 1),
                                     skip_group_check=True)
                    nc.tensor.matmul(v_ps[:], xlk,
                                     wv_all[:, k * DLOC:(k + 1) * DLOC],
                                     start=(k == 0), stop=(k == KT - 1),
                                     skip_group_check=True)
                kr = kvpool.tile([P, DLOC], BF16, name="kr", tag="kr")
                vt = kvpool.tile([P, DLOC], BF16, name="vt", tag="vt")
                if ABLATE_ROPE:
                    nc.scalar.copy(kr[:], k_ps[:])
                    nc.vector.tensor_copy(vt[:], v_ps[:])
                else:
                    cn = tpool.tile([P, DLOC], BF16, name="cn", tag="cn")
                    nc.gpsimd.dma_start(cn[:], d["cn4"][r0:r0 + 128, :])
                    sn = tpool.tile([P, DLOC], BF16, name="sn", tag="sn")
                    nc.gpsimd.dma_start(sn[:], d["sn4x"][r0:r0 + 128, :])
                    kb = tmp.tile([P, DLOC], BF16, name="kb", tag="t0")
                    nc.vector.tensor_add(kb[:], k_ps[:], bk_rep[:])
                    a2 = tmp.tile([P, DLOC], BF16, name="a2", tag="t1")
                    nc.vector.tensor_mul(a2[:], kb[:], cn[:])
                    b2 = tmp.tile([P, DLOC], BF16, name="b2", tag="t2")
                    nc.vector.tensor_mul(pair_view(b2)[:, :, 0, :],
                                         pair_view(kb)[:, :, 1, :],
                                         pair_view(sn)[:, :, 1, :])
                    nc.vector.tensor_mul(pair_view(b2)[:, :, 1, :],
                                         pair_view(kb)[:, :, 0, :],
                                         pair_view(sn)[:, :, 0, :])
                    nc.vector.tensor_add(kr[:], a2[:], b2[:])
                    nc.vector.tensor_add(vt[:], v_ps[:], bv_rep[:])

                # software-pipeline Mh by one m-tile so the PE never waits
                # on this tile's rope chain
                if mh_pend is not None:
                    emit_mh(*mh_pend)
                mh_pend = (kr, vt, mg)
        emit_mh(*mh_pend)

        m_sb = kvpool.tile([P, DLOC], BF16, name="m_sb", tag="m_sb", bufs=1)
        nc.scalar.copy(m_sb[:], mh_ps[:])

        # --- phase C: N_h = Mh_h @ woT_h (stationary shared over n) ---
        n_sb = [kvpool.tile([P, DIN], BF16, name=f"n_sb{h}", tag=f"n_sb{h}",
                            bufs=1) for h in range(NHL)]
        for h in range(NHL):
            h0, h1 = h * 128, (h + 1) * 128
            nps = [psum.tile([P, 512], F32, name="n_ps", tag="ps")
                   for _ in range(4)]
            for n in range(4):
                nc.tensor.matmul(nps[n][:], m_sb[:, h0:h1],
                                 wo_all[:, h * DIN + n * 512:h * DIN + (n + 1) * 512],
                                 start=True, stop=True, skip_group_check=True)
            for n in range(4):
                if (h + n) % 2 == 0:
                    nc.scalar.copy(n_sb[h][:, n * 512:(n + 1) * 512], nps[n][:])
                else:
                    nc.vector.tensor_copy(n_sb[h][:, n * 512:(n + 1) * 512],
                                          nps[n][:])

        # --- phase D: out = sum_h Q_h @ N_h; stationary shared over n ---
        for blk in range(NBLK):
            c0 = blk * BLK
            for m in range(4):
                ops = [psum.tile([P, 512], F32, name="o_ps", tag="ps")
                       for _ in range(4)]
                for h in range(NHL):
                    for n in range(4):
                        nc.tensor.matmul(
                            ops[n][:],
                            qtr[h][:, c0 + m * 128:c0 + (m + 1) * 128],
                            n_sb[h][:, n * 512:(n + 1) * 512],
                            start=(h == 0), stop=(h == NHL - 1),
                            skip_group_check=True)
                for n in range(4):
                    ot = osb.tile([P, 512], BF16, name="ot", tag="ot")
                    if (m + n) % 2 == 0:
                        nc.scalar.copy(ot[:], ops[n][:])
                    else:
                        nc.vector.tensor_copy(ot[:], ops[n][:])
                    nc.sync.dma_start(
                        out_d[c0 + m * 128: c0 + (m + 1) * 128,
                              n * 512:(n + 1) * 512], ot[:])

    if loop_n:
        with tc.For_i(0, loop_n, 1) as _i:
            body(False)
    else:
        for rep in range(reps):
            body(rep == 0)


def build_nc(debug_taps=False, reps=1, loop_n=0):
    global _NC_CACHE
    if _NC_CACHE is not None and reps == 1 and not loop_n:
        return _NC_CACHE
    nc = bacc.Bacc("TRN2", target_bir_lowering=False, debug=False)
    d = {
        "xT": nc.dram_tensor("xT", [DIN, S], BF16, kind="ExternalInput").ap(),
        "wqT": nc.dram_tensor("wqT", [DIN, DLOC], BF16, kind="ExternalInput").ap(),
        "wkT": nc.dram_tensor("wkT", [DIN, DLOC], BF16, kind="ExternalInput").ap(),
        "wvT": nc.dram_tensor("wvT", [DIN, DLOC], BF16, kind="ExternalInput").ap(),
        "woT": nc.dram_tensor("woT", [DLOC, DIN], BF16, kind="ExternalInput").ap(),
        "c2": nc.dram_tensor("c2", [P, S], BF16, kind="ExternalInput").ap(),
        "s2x": nc.dram_tensor("s2x", [P, S], BF16, kind="ExternalInput").ap(),
        "cn4": nc.dram_tensor("cn4", [S, DLOC], BF16, kind="ExternalInput").ap(),
        "sn4x": nc.dram_tensor("sn4x", [S, DLOC], BF16, kind="ExternalInput").ap(),
        "bqc": nc.dram_tensor("bqc", [P, NHL], BF16, kind="ExternalInput").ap(),
        "bk_rep": nc.dram_tensor("bk_rep", [P, DLOC], BF16, kind="ExternalInput").ap(),
        "bv_rep": nc.dram_tensor("bv_rep", [P, DLOC], BF16, kind="ExternalInput").ap(),
    }
    out_d = nc.dram_tensor("out", [S, DIN], BF16, kind="ExternalOutput").ap()
    with tile.TileContext(nc) as tc, ExitStack() as ctx:
        _emit(nc, tc, ctx, d, out_d, reps=reps, loop_n=loop_n)
    nc.compile()
    if reps == 1 and not loop_n:
        _NC_CACHE = nc
    return nc


def _pair_perm():
    # within each head: 64 even pair-elements then 64 odd
    idx = np.arange(DLOC).reshape(NHL, HD)
    return np.concatenate([idx[:, 0::2], idx[:, 1::2]], axis=1).reshape(-1)


def prep_in_maps(x, freqs_cos, freqs_sin, wq_w, wq_b, wk_w, wk_b,
                 wv_w, wv_b, wo_w, wo_b):
    x = np.asarray(x, np.float32)
    cos = np.asarray(freqs_cos, np.float32)
    sin = np.asarray(freqs_sin, np.float32)
    wq_w = np.asarray(wq_w, np.float32)
    wq_b = np.asarray(wq_b, np.float32)
    wk_w = np.asarray(wk_w, np.float32)
    wk_b = np.asarray(wk_b, np.float32)
    wv_w = np.asarray(wv_w, np.float32)
    wv_b = np.asarray(wv_b, np.float32)
    wo_w = np.asarray(wo_w, np.float32)

    cosT = np.ascontiguousarray(cos.T)          # [64, S]
    sinT = np.ascontiguousarray(sin.T)
    c2 = np.concatenate([cosT, cosT], axis=0).astype(NPBF)       # [128, S]
    s2x = np.concatenate([sinT, -sinT], axis=0).astype(NPBF)
    cn4 = np.tile(np.concatenate([cos, cos], axis=1), (1, NHL)).astype(NPBF)
    sn4x = np.tile(np.concatenate([sin, -sin], axis=1), (1, NHL)).astype(NPBF)

    perm = _pair_perm()
    sc = np.float32(1.0 / np.sqrt(HD))
    in_maps = []
    for c in range(NCORES):
        b, g = divmod(c, TP)
        sl = slice(g * DLOC, (g + 1) * DLOC)
        wq_p = (wq_w[sl][perm] * sc)
        bq_p = (wq_b[sl][perm] * sc)
        wk_p = wk_w[sl][perm]
        bk_p = wk_b[sl][perm]
        wv_p = wv_w[sl]
        bv_p = wv_b[sl]
        in_maps.append({
            "xT": np.ascontiguousarray(x[b].T).astype(NPBF),
            "wqT": np.ascontiguousarray(wq_p.T).astype(NPBF),
            "wkT": np.ascontiguousarray(wk_p.T).astype(NPBF),
            "wvT": np.ascontiguousarray(wv_p.T).astype(NPBF),
            "woT": np.ascontiguousarray(wo_w[:, sl].T).astype(NPBF),
            "c2": c2, "s2x": s2x, "cn4": cn4, "sn4x": sn4x,
            "bqc": np.ascontiguousarray(bq_p.reshape(NHL, P).T).astype(NPBF),
            "bk_rep": np.broadcast_to(bk_p[None, :], (P, DLOC)).astype(NPBF),
            "bv_rep": np.broadcast_to(bv_p[None, :], (P, DLOC)).astype(NPBF),
        })
    return in_maps


def assemble(results, wo_b):
    wo_b = np.asarray(wo_b, np.float32)
    out = np.zeros((B, S, DIN), np.float32)
    for c, r in enumerate(results):
        out[c // TP] += np.asarray(r["out"], np.float32)
    out += wo_b[None, None, :]
    return out


def kernel(**inputs):
    nc = build_nc()
    in_maps = prep_in_maps(
        inputs["x"], inputs["freqs_cos"], inputs["freqs_sin"],
        inputs["wq_w"], inputs["wq_b"], inputs["wk_w"], inputs["wk_b"],
        inputs["wv_w"], inputs["wv_b"], inputs["wo_w"], inputs["wo_b"])
    res = run_bass_kernel_spmd(nc, in_maps, core_ids=list(range(NCORES)))
    return assemble(res.results, inputs["wo_b"])
